# revision 2
# baseline (speedup 1.0000x reference)
"""Trainium2 Bass kernel for nn_MoDEChameleonMLP (MoDE Chameleon MLP).

Math (per token n):
  gate = x@Wg.T + delta_g(x); up = x@Wu.T + delta_u(x)
  inter = silu(gate)*up
  out  = inter@Wd.T + delta_d(inter)
where delta(v) = mask ? 2*(v@vA.T)@vB.T : 2*sum_e softmax(v@router.T)_e (v@A_e.T)@B_e.T

Implementation: token(B*S)-sharding across 8 cores (512 tokens/core, no
collectives). Each core:
  aux:    t = x@Acat.T (rank-40 LoRA bases + router logits for gate/up),
          h-chunked so it starts on partially-landed x; softmax routing +
          mask combine on DVE -> y, transposed into [40,T] form via the
          DMA XBAR (no PE, no PSUM).
  phase1: gate/up = W-stationary matmuls producing [I-part, token] tiles,
          plus one extra K=128 matmul with the (pre-scaled) LoRA B matrix
          and y -> the full delta. silu(gate)*up -> inter resident (bf16).
  phase2: down projection in 9 column-chunks; the first chunk's weight
          tile carries 44 extra columns holding the down-routing A
          matrices, so the whole down aux projection rides the existing
          matmuls (+1% rows) instead of a separate 344-matmul pass.
          Routing for the down delta reads those PSUM columns directly;
          each chunk's delta/bias stop-matmul is deferred one chunk to
          hide the routing latency.
All matmuls bf16 with fp32 PSUM accumulation. Weights are host-side
transposed/pre-tiled so every device DMA is wide contiguous lines.
"""
import os
import sys

for p in ("/root/.axon_site/_ro/trn_rl_repo", "/opt/trn_rl_repo"):
    if os.path.isdir(p) and p not in sys.path:
        sys.path.append(p)

import numpy as np
import ml_dtypes

import concourse.bass as bass  # noqa: E402
import concourse.tile as tile  # noqa: E402
from concourse import bacc, mybir  # noqa: E402
from concourse.bass_utils import run_bass_kernel_spmd  # noqa: E402

BF16 = ml_dtypes.bfloat16
BF = mybir.dt.bfloat16
F32 = mybir.dt.float32

NCORES = 8
T = 512          # tokens per core
TT = T // 128
SW = 256         # i-super width (2 x 128 psum tiles), divides 11008
E, R = 4, 8
SCALE = 2.0

# phase2 column chunking: chunk 0 carries 352 h-cols + 44 down-aux cols,
# the rest carry 468 h-cols each. 352 + 8*468 = 4096.
CH_H = [(0, 352)] + [(352 + 468 * k, 352 + 468 * (k + 1)) for k in range(8)]
CH_F = [396] + [468] * 8          # psum free width per chunk
N_CH = len(CH_H)

_nc_cache = {}


def build_kernel(H, I):
    HB, IB = H // 128, I // 128
    NS = I // SW
    NI2 = SW // 128
    XC = 4                        # xt dma chunks
    HCB = HB // XC                # h-blocks per xt chunk
    QW = 8                        # h-blocks per weight dma descriptor
    NQ = HB // QW
    wd_off = [86 * sum(CH_F[:c]) for c in range(N_CH)]
    WD_TOT = 86 * sum(CH_F)

    nc = bacc.Bacc(None, target_bir_lowering=False)
    xt_d = nc.declare_dram_parameter("xt", [128, HB, T], BF, isOutput=False)
    mask_d = nc.declare_dram_parameter("maskf", [128, 2 * TT], F32, isOutput=False)
    acall_d = nc.declare_dram_parameter("acatall", [128, HB, 88], BF, isOutput=False)
    wg_d = nc.declare_dram_parameter("wg", [NS, 128, HB, SW], BF, isOutput=False)
    wu_d = nc.declare_dram_parameter("wu", [NS, 128, HB, SW], BF, isOutput=False)
    bg_d = nc.declare_dram_parameter("bg", [NS, 128, SW], BF, isOutput=False)
    bu_d = nc.declare_dram_parameter("bu", [NS, 128, SW], BF, isOutput=False)
    wdcat_d = nc.declare_dram_parameter("wdcat", [128, WD_TOT], BF, isOutput=False)
    bdcat_d = nc.declare_dram_parameter("bdcat", [128, H], BF, isOutput=False)
    out_d = nc.declare_dram_parameter("out", [T, H], F32, isOutput=True)

    with tile.TileContext(nc) as tc:
        # wstr/wstr2/bstr2 are opened up-front so their SBUF addresses are
        # disjoint from every scoped pool: their DMAs then have no
        # address-release deps and prefetch freely across phase boundaries.
        with tc.tile_pool(name="const", bufs=1) as constp, \
             tc.tile_pool(name="wstr", bufs=11) as wstr, \
             tc.tile_pool(name="wstr2", bufs=10) as wstr2, \
             tc.tile_pool(name="bstr2", bufs=2) as bstr2:
            # ---- input DMAs, issue order matters: acall + first xt chunk
            # first so the aux pass starts ASAP; weight streams follow on
            # the same queues.
            acall_sb = constp.tile([128, HB, 88], BF)
            nc.sync.dma_start(acall_sb[:], acall_d[:])
            xtc = []
            for c in range(XC):
                xs = constp.tile([128, HCB, T], BF, tag=f"xtc{c}", name=f"xtc{c}")
                nc.sync.dma_start(xs[:], xt_d[:, c * HCB:(c + 1) * HCB, :])
                xtc.append(xs)
            mask_sb = constp.tile([128, 2 * TT], F32)
            nc.sync.dma_start(mask_sb[:], mask_d[:])

            def xth(h):
                return xtc[h // HCB][:, h % HCB, :]

            # two rotating [token, rank-pad-128] staging buffers for the
            # routing result; cols 40:128 stay zero so the transposed
            # [128, T] y tensors have zero pad rows (B matrices are
            # zero-padded there too, and 0*0 keeps NaNs out of psum).
            ytb = [constp.tile([128, 128], BF, tag=f"ytb{k}", name=f"ytb{k}")
                   for k in range(2)]
            for y in ytb:
                nc.vector.memset(y[:], 0.0)
            ygT = constp.tile([128, T], BF)
            yuT = constp.tile([128, T], BF)
            ydT = constp.tile([128, T], BF)
            inter_sb = constp.tile([128, IB, T], BF)

            def emit_route(auxtmp, ps, lo, vo, eo, t, yT, yb):
                """softmax(ps[:,lo:lo+4]) routing + mask combine -> yb
                [tok,40], then DMA-XBAR transpose into yT[:, t*128:...].
                The transpose DMA is issued from the ACT queue so it never
                blocks the Sync queue's weight streaming."""
                rmaxn = auxtmp.tile([128, 1], F32, tag="rmaxn", name=f"rx{t}")
                nc.vector.tensor_reduce(rmaxn, ps[:, lo:lo + 4],
                                        axis=mybir.AxisListType.X,
                                        op=mybir.AluOpType.max, negate=True)
                ee = auxtmp.tile([128, 4], F32, tag="ee", name=f"ee{t}")
                se = auxtmp.tile([128, 1], F32, tag="se", name=f"se{t}")
                nc.scalar.activation(ee, ps[:, lo:lo + 4],
                                     mybir.ActivationFunctionType.Exp,
                                     bias=rmaxn, accum_out=se)
                rec = auxtmp.tile([128, 1], F32, tag="rec", name=f"rc{t}")
                nc.vector.reciprocal(rec, se)
                r1m = auxtmp.tile([128, 1], F32, tag="r1m", name=f"rm{t}")
                nc.vector.tensor_tensor(r1m, rec, mask_sb[:, TT + t:TT + t + 1],
                                        mybir.AluOpType.mult)
                we = auxtmp.tile([128, 4], F32, tag="we", name=f"we{t}")
                nc.vector.tensor_scalar(we, ee, r1m, None, mybir.AluOpType.mult)
                nc.vector.tensor_scalar(yb[:, 0:8], ps[:, vo:vo + 8],
                                        mask_sb[:, t:t + 1], None,
                                        mybir.AluOpType.mult)
                for j in range(E):
                    nc.vector.tensor_scalar(yb[:, 8 + 8 * j:16 + 8 * j],
                                            ps[:, eo + 8 * j:eo + 8 * (j + 1)],
                                            we[:, j:j + 1], None,
                                            mybir.AluOpType.mult)
                nc.scalar.dma_start(yT[:, t * 128:(t + 1) * 128], yb[:],
                                    transpose=True)

            # ---- aux pass for gate/up (rank-40 projections + router
            # logits), h-chunked so matmuls start on the first xt chunk.
            with tc.tile_pool(name="auxps", bufs=1, space="PSUM") as auxps, \
                 tc.tile_pool(name="auxtmp", bufs=2) as auxtmp:
                pss = [auxps.tile([128, 512], F32, tag=f"aux{t}", name=f"aux{t}")
                       for t in range(TT)]
                for hc in range(XC):
                    for t in range(TT):
                        for h in range(hc * HCB, (hc + 1) * HCB):
                            nc.tensor.matmul(pss[t][:, :88],
                                             xth(h)[:, t * 128:(t + 1) * 128],
                                             acall_sb[:, h, :],
                                             start=(h == 0), stop=(h == HB - 1))
                for t in range(TT):
                    emit_route(auxtmp, pss[t], 80, 0, 8, t, ygT, ytb[0])
                for t in range(TT):
                    emit_route(auxtmp, pss[t], 84, 40, 48, t, yuT, ytb[1])

            # ---- phase 1: gate/up + silu*up -> inter (resident)
            with tc.tile_pool(name="bstr", bufs=3) as bstr, \
                 tc.tile_pool(name="etmp", bufs=3) as etmp, \
                 tc.tile_pool(name="mps", bufs=2, space="PSUM") as mps:
                for s in range(NS):
                    psgu = []
                    for proj, w_dram, b_dram in (("g", wg_d, bg_d), ("u", wu_d, bu_d)):
                        wt = []
                        for q in range(NQ):
                            wq = wstr.tile([128, QW, SW], BF, tag="wt",
                                           name=f"w{proj}{s}_{q}")
                            nc.sync.dma_start(wq[:], w_dram[s, :, q * QW:(q + 1) * QW, :])
                            wt.append(wq)
                        bt = bstr.tile([128, SW], BF, tag="bt", name=f"b{proj}{s}")
                        nc.sync.dma_start(bt[:], b_dram[s])
                        yT = ygT if proj == "g" else yuT
                        pss = [mps.tile([128, 512], F32, tag=f"p{proj}{i2}",
                                        name=f"p{proj}{s}_{i2}") for i2 in range(NI2)]
                        for h in range(HB):
                            for i2 in range(NI2):
                                nc.tensor.matmul(pss[i2],
                                                 wt[h // QW][:, h % QW,
                                                             i2 * 128:(i2 + 1) * 128],
                                                 xth(h),
                                                 start=(h == 0), stop=False)
                        for i2 in range(NI2):
                            nc.tensor.matmul(pss[i2], bt[:, i2 * 128:(i2 + 1) * 128],
                                             yT[:], start=False, stop=True)
                        psgu.append(pss)
                    for i2 in range(NI2):
                        i = s * NI2 + i2
                        st = etmp.tile([128, T], F32, tag="silu", name=f"si{s}_{i2}")
                        nc.scalar.activation(st[:], psgu[0][i2][:, :T],
                                             mybir.ActivationFunctionType.Silu)
                        nc.vector.tensor_tensor(inter_sb[:, i, :], st[:],
                                                psgu[1][i2][:, :T],
                                                mybir.AluOpType.mult)

            # ---- phase 2: down projection in N_CH column chunks. Chunk 0
            # carries the 44 down-aux columns; its delta/bias stop-matmul
            # (and chunk 1's) are emitted after chunk 1's i-loop so the
            # routing latency hides under real matmul work.
            with tc.tile_pool(name="ops", bufs=2, space="PSUM") as ops, \
                 tc.tile_pool(name="ost", bufs=3) as ost, \
                 tc.tile_pool(name="auxtmp2", bufs=2) as auxtmp2:
                pending = []

                def finish(c, pso, bdt):
                    a, b = CH_H[c]
                    wh = b - a
                    for t in range(TT):
                        nc.tensor.matmul(pso[t][:, :wh],
                                         ydT[:, t * 128:(t + 1) * 128],
                                         bdt[:, :wh], start=False, stop=True)
                        osb = ost.tile([128, 468], F32, tag="os", name=f"os{c}_{t}")
                        nc.vector.tensor_copy(osb[:, :wh], pso[t][:, :wh])
                        nc.sync.dma_start(
                            out_d[t * 128:(t + 1) * 128, a:b], osb[:, :wh])

                for c in range(N_CH):
                    a, b = CH_H[c]
                    wh, wc = b - a, CH_F[c]
                    bdt = bstr2.tile([128, 468], BF, tag="bd2", name=f"bd{c}")
                    nc.sync.dma_start(bdt[:, :wh], bdcat_d[:, a:b])
                    pso = [ops.tile([128, 512], F32, tag=f"po{t}",
                                    name=f"po{c}_{t}") for t in range(TT)]
                    for ip in range(IB // 2):
                        wdt = wstr2.tile([128, 2 * 468], BF, tag="wd2",
                                         name=f"wd{c}_{ip}")
                        nc.sync.dma_start(
                            wdt[:, :2 * wc],
                            wdcat_d[:, wd_off[c] + ip * 2 * wc:
                                    wd_off[c] + (ip + 1) * 2 * wc])
                        for j in range(2):
                            i = 2 * ip + j
                            for t in range(TT):
                                nc.tensor.matmul(
                                    pso[t][:, :wc],
                                    inter_sb[:, i, t * 128:(t + 1) * 128],
                                    wdt[:, j * wc:(j + 1) * wc],
                                    start=(i == 0), stop=False)
                    if c == 0:
                        # down-routing straight from the psum aux columns
                        for t in range(TT):
                            emit_route(auxtmp2, pso[t], 352, 356, 364, t,
                                       ydT, ytb[t % 2])
                        pending.append((c, pso, bdt))
                    elif c == 1:
                        pc, ppso, pbdt = pending.pop()
                        finish(pc, ppso, pbdt)
                        finish(c, pso, bdt)
                    else:
                        finish(c, pso, bdt)
    nc.finalize()
    return nc


def get_nc(H, I):
    key = (H, I)
    if key not in _nc_cache:
        _nc_cache[key] = build_kernel(H, I)
    return _nc_cache[key]


def _prep_weights(Wg, Wu, Wd, va_gate_A, va_gate_B, va_up_A, va_up_B,
                  va_down_A, va_down_B, router_gate, tm_gate_A, tm_gate_B,
                  router_up, tm_up_A, tm_up_B, router_down, tm_down_A, tm_down_B):
    I, H = Wg.shape
    HB, IB = H // 128, I // 128
    NS = I // SW

    def tile_w_ih(W):  # [I,H] -> [NS,128,HB,SW]; w[s,p,h,c]=W[s*SW+c, h*128+p]
        return np.ascontiguousarray(
            W.reshape(NS, SW, HB, 128).transpose(0, 3, 2, 1)).astype(BF16)

    def tile_bcat(vB, tB, rows):  # -> [nblk,128,blk]; padded 2*[vB|tB_e].T
        out_dim = vB.shape[0]
        Bcat = np.concatenate([vB] + [tB[e] for e in range(E)], axis=1)  # [out,40]
        Bp = np.zeros((128, out_dim), np.float32)
        Bp[:40, :] = SCALE * Bcat.T
        blk = out_dim // rows
        return np.ascontiguousarray(
            Bp.reshape(128, rows, blk).transpose(1, 0, 2)).astype(BF16)

    A_all = np.concatenate([va_gate_A, tm_gate_A.reshape(E * R, H),
                            va_up_A, tm_up_A.reshape(E * R, H),
                            router_gate, router_up], axis=0)  # [88,H]
    acatall = np.ascontiguousarray(
        A_all.T.reshape(HB, 128, 88).transpose(1, 0, 2)).astype(BF16)
    A_d = np.concatenate([router_down, va_down_A,
                          tm_down_A.reshape(E * R, I)], axis=0)  # [44,I]

    # down weights in column chunks; chunk 0 carries the down-aux columns
    parts = []
    for c, (a, b) in enumerate(CH_H):
        cols = Wd[a:b, :]                                    # [wh, I]
        if c == 0:
            cols = np.concatenate([cols, A_d], axis=0)       # [wh+44, I]
        wc = cols.shape[0]
        t = cols.T.reshape(IB, 128, wc).transpose(1, 0, 2)   # [128,IB,wc]
        parts.append(t.reshape(128, IB * wc))
    wdcat = np.ascontiguousarray(np.concatenate(parts, axis=1)).astype(BF16)

    Bcat_d = np.concatenate([va_down_B] + [tm_down_B[e] for e in range(E)],
                            axis=1)                          # [H,40]
    bdcat = np.zeros((128, H), np.float32)
    bdcat[:40, :] = SCALE * Bcat_d.T
    bdcat = np.ascontiguousarray(bdcat).astype(BF16)

    return {
        "acatall": acatall,
        "wg": tile_w_ih(Wg),
        "wu": tile_w_ih(Wu),
        "bg": tile_bcat(va_gate_B, tm_gate_B, NS),
        "bu": tile_bcat(va_up_B, tm_up_B, NS),
        "wdcat": wdcat,
        "bdcat": bdcat,
    }


def _prep_core_inputs(x, image_mask, weights, n_cores):
    Bb, S, H = x.shape
    HB = H // 128
    xf = np.asarray(x, np.float32).reshape(-1, H)
    m = np.asarray(image_mask).reshape(-1).astype(np.float32)
    in_maps = []
    for c in range(n_cores):
        sh = xf[c * T:(c + 1) * T]                      # [T,H]
        xt = np.ascontiguousarray(
            sh.T.reshape(HB, 128, T).transpose(1, 0, 2)).astype(BF16)
        mc = m[c * T:(c + 1) * T].reshape(TT, 128).T    # [128,TT]
        maskf = np.ascontiguousarray(
            np.concatenate([mc, 1.0 - mc], axis=1)).astype(np.float32)
        in_maps.append({"xt": xt, "maskf": maskf, **weights})
    return in_maps


def run(x, image_mask, weights_raw, trace=False):
    Bb, S, H = x.shape
    I = weights_raw["Wg"].shape[0]
    nc = get_nc(H, I)
    weights = _prep_weights(**weights_raw)
    in_maps = _prep_core_inputs(x, image_mask, weights, NCORES)
    res = run_bass_kernel_spmd(nc, in_maps, list(range(NCORES)), trace=trace)
    out = np.concatenate([r["out"] for r in res.results], axis=0)
    return out.reshape(Bb, S, H).astype(np.float32), res


def kernel(x, image_mask, Wg, Wu, Wd,
           va_gate_A, va_gate_B, va_up_A, va_up_B, va_down_A, va_down_B,
           router_gate, tm_gate_A, tm_gate_B,
           router_up, tm_up_A, tm_up_B,
           router_down, tm_down_A, tm_down_B):
    weights_raw = dict(
        Wg=np.asarray(Wg, np.float32), Wu=np.asarray(Wu, np.float32),
        Wd=np.asarray(Wd, np.float32),
        va_gate_A=np.asarray(va_gate_A), va_gate_B=np.asarray(va_gate_B),
        va_up_A=np.asarray(va_up_A), va_up_B=np.asarray(va_up_B),
        va_down_A=np.asarray(va_down_A), va_down_B=np.asarray(va_down_B),
        router_gate=np.asarray(router_gate), tm_gate_A=np.asarray(tm_gate_A),
        tm_gate_B=np.asarray(tm_gate_B),
        router_up=np.asarray(router_up), tm_up_A=np.asarray(tm_up_A),
        tm_up_B=np.asarray(tm_up_B),
        router_down=np.asarray(router_down), tm_down_A=np.asarray(tm_down_A),
        tm_down_B=np.asarray(tm_down_B),
    )
    out, _ = run(np.asarray(x), np.asarray(image_mask), weights_raw, trace=False)
    return out


# revision 8
# speedup vs baseline: 1.0255x; 1.0255x over previous
"""Trainium2 Bass kernel for nn_MoDEChameleonMLP (MoDE Chameleon MLP).

Math (per token n):
  gate = x@Wg.T + delta_g(x); up = x@Wu.T + delta_u(x)
  inter = silu(gate)*up
  out  = inter@Wd.T + delta_d(inter)
where delta(v) = mask ? 2*(v@vA.T)@vB.T : 2*sum_e softmax(v@router.T)_e (v@A_e.T)@B_e.T

Implementation: token(B*S)-sharding across 8 cores (512 tokens/core, no
collectives). Each core:
  aux:    t = x@Acat.T (rank-40 LoRA bases + router logits for gate/up),
          h-chunked over the streaming x DMA, all four token-tiles packed
          into a single PSUM bank (one start zeroes the bank; disjoint
          column ranges accumulate independently). Routing (softmax +
          mask combine) on DVE, transposed to [40,T] via PE+identity.
  phase1: gate/up = W-stationary matmuls producing [I-part, token] tiles,
          plus one extra K=128 matmul with the (pre-scaled) LoRA B matrix
          and y -> the full delta. s=0's weight DMAs and gate psum tags
          are pre-reserved before the aux scope so neither SBUF nor PSUM
          address reuse serializes phase1 behind the routing chains.
  phase2: down projection in 9 column-chunks; the first chunk's weight
          tile carries 44 extra columns holding the down-routing A
          matrices, so the whole down aux projection rides the existing
          matmuls (+1% rows) instead of a separate 344-matmul pass.
          Down-routing reads those PSUM columns directly; its transposes
          borrow the second generation of the output psum rings.
All matmuls bf16 with fp32 PSUM accumulation. Weights are host-side
transposed/pre-tiled so every device DMA is wide contiguous lines.
"""
import os
import sys

for p in ("/root/.axon_site/_ro/trn_rl_repo", "/opt/trn_rl_repo"):
    if os.path.isdir(p) and p not in sys.path:
        sys.path.append(p)

import numpy as np
import ml_dtypes

import concourse.bass as bass  # noqa: E402
import concourse.tile as tile  # noqa: E402
from concourse import bacc, mybir  # noqa: E402
from concourse.bass_utils import run_bass_kernel_spmd  # noqa: E402
from concourse.masks import make_identity  # noqa: E402

BF16 = ml_dtypes.bfloat16
BF = mybir.dt.bfloat16
F32 = mybir.dt.float32

NCORES = 8
T = 512          # tokens per core
TT = T // 128
SW = 256         # i-super width (2 x 128 psum tiles), divides 11008
E, R = 4, 8
SCALE = 2.0

# phase2 column chunking: chunk 0 carries 352 h-cols + 44 down-aux cols,
# the rest carry 468 h-cols each. 352 + 8*468 = 4096.
CH_H = [(0, 352)] + [(352 + 468 * k, 352 + 468 * (k + 1)) for k in range(8)]
CH_F = [396] + [468] * 8          # psum free width per chunk
N_CH = len(CH_H)

_nc_cache = {}


def build_kernel(H, I):
    HB, IB = H // 128, I // 128
    NS = I // SW
    NI2 = SW // 128
    XC = 4                        # xt dma chunks
    HCB = HB // XC                # h-blocks per xt chunk
    QW = 8                        # h-blocks per weight dma descriptor
    NQ = HB // QW
    wd_off = [86 * sum(CH_F[:c]) for c in range(N_CH)]
    WD_TOT = 86 * sum(CH_F)

    nc = bacc.Bacc(None, target_bir_lowering=False)
    xt_d = nc.declare_dram_parameter("xt", [128, HB, T], BF, isOutput=False)
    mask_d = nc.declare_dram_parameter("maskf", [128, 2 * TT], F32, isOutput=False)
    acall_d = nc.declare_dram_parameter("acatall", [128, HB, 88], BF, isOutput=False)
    wg_d = nc.declare_dram_parameter("wg", [NS, 128, HB, SW], BF, isOutput=False)
    wu_d = nc.declare_dram_parameter("wu", [NS, 128, HB, SW], BF, isOutput=False)
    bg_d = nc.declare_dram_parameter("bg", [NS, 128, SW], BF, isOutput=False)
    bu_d = nc.declare_dram_parameter("bu", [NS, 128, SW], BF, isOutput=False)
    wdcat_d = nc.declare_dram_parameter("wdcat", [128, WD_TOT], BF, isOutput=False)
    bdcat_d = nc.declare_dram_parameter("bdcat", [128, H], BF, isOutput=False)
    out_d = nc.declare_dram_parameter("out", [T, H], F32, isOutput=True)

    with tile.TileContext(nc) as tc:
        # wstr/wstr2/bstr2 are opened up-front so their SBUF addresses are
        # disjoint from every scoped pool: their DMAs then have no
        # address-release deps and prefetch freely across phase boundaries.
        with tc.tile_pool(name="const", bufs=1) as constp, \
             tc.tile_pool(name="wstr", bufs=11) as wstr, \
             tc.tile_pool(name="wstr2", bufs=10) as wstr2, \
             tc.tile_pool(name="bstr2", bufs=2) as bstr2:
            # ---- input DMAs, issue order matters: acall + xt chunks first
            # so the aux pass starts ASAP; s=0 weights follow immediately.
            acall_sb = constp.tile([128, HB, 88], BF)
            nc.sync.dma_start(acall_sb[:], acall_d[:])
            xtc = []
            for c in range(XC):
                xs = constp.tile([128, HCB, T], BF, tag=f"xtc{c}", name=f"xtc{c}")
                nc.sync.dma_start(xs[:], xt_d[:, c * HCB:(c + 1) * HCB, :])
                xtc.append(xs)
            mask_sb = constp.tile([128, 2 * TT], F32)
            nc.sync.dma_start(mask_sb[:], mask_d[:])

            def xth(h):
                return xtc[h // HCB][:, h % HCB, :]

            ident = constp.tile([128, 128], BF)
            make_identity(nc, ident)
            ygT = constp.tile([128, T], BF)
            yuT = constp.tile([128, T], BF)
            ydT = constp.tile([128, T], BF)
            for y in (ygT, yuT, ydT):
                nc.vector.memset(y[:], 0.0)
            inter_sb = constp.tile([128, IB, T], BF)

            def emit_route(tpp, tptag, tpbufs, auxtmp, ps, lo, vo, eo, t, yT):
                """softmax(ps[:,lo:lo+4]) routing + mask combine -> y, then
                transpose y[128,40] into yT[0:40, t*128:(t+1)*128] via PE."""
                rmaxn = auxtmp.tile([128, 1], F32, tag="rmaxn", name=f"rx{t}")
                nc.vector.tensor_reduce(rmaxn, ps[:, lo:lo + 4],
                                        axis=mybir.AxisListType.X,
                                        op=mybir.AluOpType.max, negate=True)
                ee = auxtmp.tile([128, 4], F32, tag="ee", name=f"ee{t}")
                se = auxtmp.tile([128, 1], F32, tag="se", name=f"se{t}")
                nc.scalar.activation(ee, ps[:, lo:lo + 4],
                                     mybir.ActivationFunctionType.Exp,
                                     bias=rmaxn, accum_out=se)
                rec = auxtmp.tile([128, 1], F32, tag="rec", name=f"rc{t}")
                nc.vector.reciprocal(rec, se)
                r1m = auxtmp.tile([128, 1], F32, tag="r1m", name=f"rm{t}")
                nc.vector.tensor_tensor(r1m, rec, mask_sb[:, TT + t:TT + t + 1],
                                        mybir.AluOpType.mult)
                we = auxtmp.tile([128, 4], F32, tag="we", name=f"we{t}")
                nc.vector.tensor_scalar(we, ee, r1m, None, mybir.AluOpType.mult)
                yt = auxtmp.tile([128, 40], BF, tag="yt", name=f"yt{t}")
                nc.vector.tensor_scalar(yt[:, 0:8], ps[:, vo:vo + 8],
                                        mask_sb[:, t:t + 1], None,
                                        mybir.AluOpType.mult)
                for j in range(E):
                    nc.vector.tensor_scalar(yt[:, 8 + 8 * j:16 + 8 * j],
                                            ps[:, eo + 8 * j:eo + 8 * (j + 1)],
                                            we[:, j:j + 1], None,
                                            mybir.AluOpType.mult)
                tp = tpp.tile([128, 128], BF, tag=tptag, name=f"tp{t}",
                              bufs=tpbufs)
                nc.tensor.transpose(tp[:40, :], yt[:], ident)
                nc.vector.tensor_copy(yT[0:40, t * 128:(t + 1) * 128], tp[:40, :])

            with tc.tile_pool(name="bstr", bufs=3) as bstr:

                def proj_weight_dmas(s, proj, w_dram, b_dram):
                    wt = []
                    for q in range(NQ):
                        wq = wstr.tile([128, QW, SW], BF, tag="wt",
                                       name=f"w{proj}{s}_{q}")
                        nc.sync.dma_start(wq[:], w_dram[s, :, q * QW:(q + 1) * QW, :])
                        wt.append(wq)
                    bt = bstr.tile([128, SW], BF, tag="bt", name=f"b{proj}{s}")
                    nc.sync.dma_start(bt[:], b_dram[s])
                    return wt, bt

                def proj_mains(pss, wt):
                    for h in range(HB):
                        for i2 in range(NI2):
                            nc.tensor.matmul(pss[i2],
                                             wt[h // QW][:, h % QW,
                                                         i2 * 128:(i2 + 1) * 128],
                                             xth(h),
                                             start=(h == 0), stop=False)

                def proj_delta(pss, bt, yT):
                    for i2 in range(NI2):
                        nc.tensor.matmul(pss[i2], bt[:, i2 * 128:(i2 + 1) * 128],
                                         yT[:], start=False, stop=True)

                # s=0 weight DMAs issued now (before any scoped pool opens)
                # so the Sync queue streams weights from t~10us, and the
                # wstr/bstr tags claim SBUF below the aux scratch.
                wt_g0, bt_g0 = proj_weight_dmas(0, "g", wg_d, bg_d)
                wt_u0, bt_u0 = proj_weight_dmas(0, "u", wu_d, bu_d)

                # ---- scope A: aux pass (all four token-tiles in ONE psum
                # bank) + the WHOLE s=0 iteration, fully self-contained so
                # nothing escapes the scope: 2+2+1+2 = 7 psum banks, freed
                # before the main 8-bank ring opens.
                with tc.tile_pool(name="auxps", bufs=1, space="PSUM") as auxps, \
                     tc.tile_pool(name="auxtmp", bufs=2) as auxtmp:
                    psg0 = [auxps.tile([128, 512], F32, tag=f"sg{i2}",
                                       name=f"pg0_{i2}") for i2 in range(NI2)]
                    psu0 = [auxps.tile([128, 512], F32, tag=f"su{i2}",
                                       name=f"pu0_{i2}") for i2 in range(NI2)]
                    auxpk = auxps.tile([128, 512], F32, tag="aux", name="auxpk")
                    for hc in range(XC):
                        for t in range(TT):
                            for h in range(hc * HCB, (hc + 1) * HCB):
                                nc.tensor.matmul(
                                    auxpk[:, t * 128:t * 128 + 88],
                                    xth(h)[:, t * 128:(t + 1) * 128],
                                    acall_sb[:, h, :],
                                    start=(t == 0 and h == 0),
                                    stop=(t == TT - 1 and h == HB - 1))
                    # s0 gate mains run while the routing chains execute on
                    # DVE/ACT; the PE transposes land right after them.
                    proj_mains(psg0, wt_g0)
                    for t in range(TT):
                        emit_route(auxps, "tp", 2, auxtmp,
                                   auxpk[:, t * 128:(t + 1) * 128],
                                   80, 0, 8, t, ygT)
                    proj_delta(psg0, bt_g0, ygT)
                    for t in range(TT):
                        emit_route(auxps, "tp", 2, auxtmp,
                                   auxpk[:, t * 128:(t + 1) * 128],
                                   84, 40, 48, t, yuT)
                    proj_mains(psu0, wt_u0)
                    proj_delta(psu0, bt_u0, yuT)
                    for i2 in range(NI2):
                        st0 = auxtmp.tile([128, T], F32, tag="silu0",
                                          name=f"si0_{i2}")
                        nc.scalar.activation(st0[:], psg0[i2][:, :T],
                                             mybir.ActivationFunctionType.Silu)
                        nc.vector.tensor_tensor(inter_sb[:, i2, :], st0[:],
                                                psu0[i2][:, :T],
                                                mybir.AluOpType.mult)

                # ---- scope B: the standard 8-bank double-buffered ring for
                # s=1..NS-1. Writers into banks that alias freed scope-A
                # tiles wait on their pending readers, which are long done.
                with tc.tile_pool(name="etmp", bufs=3) as etmp, \
                     tc.tile_pool(name="mps", bufs=2, space="PSUM") as mps:

                    def proj_silu(s, psg, psu):
                        for i2 in range(NI2):
                            i = s * NI2 + i2
                            st = etmp.tile([128, T], F32, tag="silu",
                                           name=f"si{s}_{i2}")
                            nc.scalar.activation(st[:], psg[i2][:, :T],
                                                 mybir.ActivationFunctionType.Silu)
                            nc.vector.tensor_tensor(inter_sb[:, i, :], st[:],
                                                    psu[i2][:, :T],
                                                    mybir.AluOpType.mult)

                    for s in range(1, NS):
                        psgu = []
                        for proj, w_dram, b_dram, tg in (("g", wg_d, bg_d, "pg"),
                                                         ("u", wu_d, bu_d, "pu")):
                            wt, bt = proj_weight_dmas(s, proj, w_dram, b_dram)
                            yT = ygT if proj == "g" else yuT
                            pss = [mps.tile([128, 512], F32, tag=f"{tg}{i2}",
                                            name=f"p{proj}{s}_{i2}")
                                   for i2 in range(NI2)]
                            proj_mains(pss, wt)
                            proj_delta(pss, bt, yT)
                            psgu.append(pss)
                        proj_silu(s, psgu[0], psgu[1])

            # ---- phase 2: down projection in N_CH column chunks. Chunk 0
            # carries the 44 down-aux columns; routing runs right after its
            # i-loop (PE pays ~3us once for the DVE chain latency).
            with tc.tile_pool(name="ops", bufs=2, space="PSUM") as ops, \
                 tc.tile_pool(name="ost", bufs=3) as ost, \
                 tc.tile_pool(name="auxtmp2", bufs=2) as auxtmp2:

                def finish(c, pso, bdt):
                    a, b = CH_H[c]
                    wh = b - a
                    for t in range(TT):
                        nc.tensor.matmul(pso[t][:, :wh],
                                         ydT[:, t * 128:(t + 1) * 128],
                                         bdt[:, :wh], start=False, stop=True)
                        osb = ost.tile([128, 468], F32, tag="os", name=f"os{c}_{t}")
                        nc.vector.tensor_copy(osb[:, :wh], pso[t][:, :wh])
                        nc.sync.dma_start(
                            out_d[t * 128:(t + 1) * 128, a:b], osb[:, :wh])

                for c in range(N_CH):
                    a, b = CH_H[c]
                    wh, wc = b - a, CH_F[c]
                    bdt = bstr2.tile([128, 468], BF, tag="bd2", name=f"bd{c}")
                    nc.sync.dma_start(bdt[:, :wh], bdcat_d[:, a:b])
                    pso = [ops.tile([128, 512], F32, tag=f"po{t}",
                                    name=f"po{c}_{t}") for t in range(TT)]
                    for ip in range(IB // 2):
                        wdt = wstr2.tile([128, 2 * 468], BF, tag="wd2",
                                         name=f"wd{c}_{ip}")
                        nc.sync.dma_start(
                            wdt[:, :2 * wc],
                            wdcat_d[:, wd_off[c] + ip * 2 * wc:
                                    wd_off[c] + (ip + 1) * 2 * wc])
                        for j in range(2):
                            i = 2 * ip + j
                            for t in range(TT):
                                nc.tensor.matmul(
                                    pso[t][:, :wc],
                                    inter_sb[:, i, t * 128:(t + 1) * 128],
                                    wdt[:, j * wc:(j + 1) * wc],
                                    start=(i == 0), stop=False)
                    if c == 0:
                        # down-routing straight from the psum aux columns;
                        # the transposes borrow gen-2 of the po tag rings.
                        for t in range(TT):
                            emit_route(ops, f"po{t}", 2, auxtmp2, pso[t],
                                       352, 356, 364, t, ydT)
                    finish(c, pso, bdt)
    nc.finalize()
    return nc


def get_nc(H, I):
    key = (H, I)
    if key not in _nc_cache:
        _nc_cache[key] = build_kernel(H, I)
    return _nc_cache[key]


def _prep_weights(Wg, Wu, Wd, va_gate_A, va_gate_B, va_up_A, va_up_B,
                  va_down_A, va_down_B, router_gate, tm_gate_A, tm_gate_B,
                  router_up, tm_up_A, tm_up_B, router_down, tm_down_A, tm_down_B):
    I, H = Wg.shape
    HB, IB = H // 128, I // 128
    NS = I // SW

    def tile_w_ih(W):  # [I,H] -> [NS,128,HB,SW]; w[s,p,h,c]=W[s*SW+c, h*128+p]
        return np.ascontiguousarray(
            W.reshape(NS, SW, HB, 128).transpose(0, 3, 2, 1)).astype(BF16)

    def tile_bcat(vB, tB, rows):  # -> [nblk,128,blk]; padded 2*[vB|tB_e].T
        out_dim = vB.shape[0]
        Bcat = np.concatenate([vB] + [tB[e] for e in range(E)], axis=1)  # [out,40]
        Bp = np.zeros((128, out_dim), np.float32)
        Bp[:40, :] = SCALE * Bcat.T
        blk = out_dim // rows
        return np.ascontiguousarray(
            Bp.reshape(128, rows, blk).transpose(1, 0, 2)).astype(BF16)

    A_all = np.concatenate([va_gate_A, tm_gate_A.reshape(E * R, H),
                            va_up_A, tm_up_A.reshape(E * R, H),
                            router_gate, router_up], axis=0)  # [88,H]
    acatall = np.ascontiguousarray(
        A_all.T.reshape(HB, 128, 88).transpose(1, 0, 2)).astype(BF16)
    A_d = np.concatenate([router_down, va_down_A,
                          tm_down_A.reshape(E * R, I)], axis=0)  # [44,I]

    # down weights in column chunks; chunk 0 carries the down-aux columns
    parts = []
    for c, (a, b) in enumerate(CH_H):
        cols = Wd[a:b, :]                                    # [wh, I]
        if c == 0:
            cols = np.concatenate([cols, A_d], axis=0)       # [wh+44, I]
        wc = cols.shape[0]
        t = cols.T.reshape(IB, 128, wc).transpose(1, 0, 2)   # [128,IB,wc]
        parts.append(t.reshape(128, IB * wc))
    wdcat = np.ascontiguousarray(np.concatenate(parts, axis=1)).astype(BF16)

    Bcat_d = np.concatenate([va_down_B] + [tm_down_B[e] for e in range(E)],
                            axis=1)                          # [H,40]
    bdcat = np.zeros((128, H), np.float32)
    bdcat[:40, :] = SCALE * Bcat_d.T
    bdcat = np.ascontiguousarray(bdcat).astype(BF16)

    return {
        "acatall": acatall,
        "wg": tile_w_ih(Wg),
        "wu": tile_w_ih(Wu),
        "bg": tile_bcat(va_gate_B, tm_gate_B, NS),
        "bu": tile_bcat(va_up_B, tm_up_B, NS),
        "wdcat": wdcat,
        "bdcat": bdcat,
    }


def _prep_core_inputs(x, image_mask, weights, n_cores):
    Bb, S, H = x.shape
    HB = H // 128
    xf = np.asarray(x, np.float32).reshape(-1, H)
    m = np.asarray(image_mask).reshape(-1).astype(np.float32)
    in_maps = []
    for c in range(n_cores):
        sh = xf[c * T:(c + 1) * T]                      # [T,H]
        xt = np.ascontiguousarray(
            sh.T.reshape(HB, 128, T).transpose(1, 0, 2)).astype(BF16)
        mc = m[c * T:(c + 1) * T].reshape(TT, 128).T    # [128,TT]
        maskf = np.ascontiguousarray(
            np.concatenate([mc, 1.0 - mc], axis=1)).astype(np.float32)
        in_maps.append({"xt": xt, "maskf": maskf, **weights})
    return in_maps


def run(x, image_mask, weights_raw, trace=False):
    Bb, S, H = x.shape
    I = weights_raw["Wg"].shape[0]
    nc = get_nc(H, I)
    weights = _prep_weights(**weights_raw)
    in_maps = _prep_core_inputs(x, image_mask, weights, NCORES)
    res = run_bass_kernel_spmd(nc, in_maps, list(range(NCORES)), trace=trace)
    out = np.concatenate([r["out"] for r in res.results], axis=0)
    return out.reshape(Bb, S, H).astype(np.float32), res


def kernel(x, image_mask, Wg, Wu, Wd,
           va_gate_A, va_gate_B, va_up_A, va_up_B, va_down_A, va_down_B,
           router_gate, tm_gate_A, tm_gate_B,
           router_up, tm_up_A, tm_up_B,
           router_down, tm_down_A, tm_down_B):
    weights_raw = dict(
        Wg=np.asarray(Wg, np.float32), Wu=np.asarray(Wu, np.float32),
        Wd=np.asarray(Wd, np.float32),
        va_gate_A=np.asarray(va_gate_A), va_gate_B=np.asarray(va_gate_B),
        va_up_A=np.asarray(va_up_A), va_up_B=np.asarray(va_up_B),
        va_down_A=np.asarray(va_down_A), va_down_B=np.asarray(va_down_B),
        router_gate=np.asarray(router_gate), tm_gate_A=np.asarray(tm_gate_A),
        tm_gate_B=np.asarray(tm_gate_B),
        router_up=np.asarray(router_up), tm_up_A=np.asarray(tm_up_A),
        tm_up_B=np.asarray(tm_up_B),
        router_down=np.asarray(router_down), tm_down_A=np.asarray(tm_down_A),
        tm_down_B=np.asarray(tm_down_B),
    )
    out, _ = run(np.asarray(x), np.asarray(image_mask), weights_raw, trace=False)
    return out
